# revision 1
# baseline (speedup 1.0000x reference)
"""LDPC belief-propagation kernel for Trainium2 (8 NeuronCores, data-parallel).

Math (per batch row, H fixed [3,7], 12 edges, check-major edge order):
  lu_e  = ln|tanh(m_e/2)|           = ln(1-z) - ln(1+z),  z = exp(-|m_e|)
  S_c   = sum_{e in check c} lu_e
  d_e   = S_c - lu_e                (== s_upd, <= 0)
  mag_e = -ln tanh(|d_e|/2)         = ln(1+u) - ln(1-u),  u = exp(d_e)
  sgn_e = prod_{e' in c} sign(m_{e'}) * sign(m_e)    (leave-one-out, +-1)
  c2v_e = mag_e * sgn_e
  new_llr_v = llr_v + sum_{c contains v} c2v_{c,v}
  m'_e  = new_llr_v - c2v_e
Only Exp/Ln/Abs/Sign activations -> one ACT table set, no table switches.
Edges of degree-1 variables (e0,e4,e8) carry constant messages == llr: their
lu/sign are computed once; per-iteration transcendentals cover only the 9
dynamic edges, and deg-1 new_llr terms are added only on the last iteration.
Batch is split into chunks so ACT/DVE/GPSIMD/DMA pipeline across chunks.
"""

import numpy as np

_CACHE = {}

NCORES = 8
P = 128      # partitions
CHUNKS = 2   # batch sub-chunks per core (pipeline depth)


def _build(Bc, iters):
    import contextlib

    import concourse.bass as bass
    import concourse.tile as tile
    from concourse import mybir
    from concourse.alu_op_type import AluOpType as Op

    F = mybir.ActivationFunctionType
    W = Bc // P // CHUNKS  # free columns per partition per chunk
    f32 = mybir.dt.float32

    nc = bass.Bass("TRN2", target_bir_lowering=False, debug=False,
                   num_devices=1)
    llr_d = nc.dram_tensor("llr", [Bc, 7], f32, kind="ExternalInput")
    out_d = nc.dram_tensor("out", [Bc, 7], f32, kind="ExternalOutput")

    def sub(t, off, dims):
        a = t[:] if callable(getattr(t, "__getitem__", None)) else t
        return bass.AP(tensor=a.tensor, offset=a.offset + off,
                       ap=[list(a.ap[0])] + [list(d) for d in dims])

    with tile.TileContext(nc) as tc:
        ctx = contextlib.ExitStack()
        with ctx:
            keep = ctx.enter_context(tc.tile_pool(name="keep", bufs=1))
            work = ctx.enter_context(tc.tile_pool(name="work", bufs=2))

            def K(name, k):
                return keep.tile([P, W * k], f32, tag=name, name=name)

            CB = keep.tile([P, 1], f32, tag="CB", name="CB")
            nc.vector.memset(CB, 1e-38)
            CB2 = keep.tile([P, 1], f32, tag="CB2", name="CB2")
            nc.vector.memset(CB2, 0.99999994)

            # per-chunk persistent state
            LLRs = [K(f"LLR{c}", 7) for c in range(CHUNKS)]
            Ms   = [K(f"M{c}", 12) for c in range(CHUNKS)]
            LUs  = [K(f"LU{c}", 12) for c in range(CHUNKS)]
            SGs  = [K(f"SG{c}", 12) for c in range(CHUNKS)]
            NLs  = [K(f"NL{c}", 7) for c in range(CHUNKS)]

            act = nc.scalar.activation
            vec = nc.vector
            gps = nc.gpsimd

            def g12(t):
                return sub(t, 0, [[12, W], [4, 3], [1, 4]])

            def dyn9(t):
                return sub(t, 1, [[12, W], [4, 3], [1, 3]])

            llr_ap = llr_d.ap().rearrange("(c p w) v -> c p (w v)", c=CHUNKS, p=P)
            out_ap = out_d.ap().rearrange("(c p w) v -> c p (w v)", c=CHUNKS, p=P)

            for c in range(CHUNKS):
                LLR, M = LLRs[c], Ms[c]
                nc.sync.dma_start(out=LLR[:], in_=llr_ap[c])
                vec.tensor_copy(sub(M, 0, [[12, W], [1, 4]]),
                                sub(LLR, 0, [[7, W], [2, 4]]))
                vec.tensor_copy(sub(M, 4, [[12, W], [1, 2]]),
                                sub(LLR, 1, [[7, W], [1, 2]]))
                vec.tensor_copy(sub(M, 6, [[12, W], [1, 2]]),
                                sub(LLR, 5, [[7, W], [1, 2]]))
                vec.tensor_copy(sub(M, 8, [[12, W], [1, 4]]),
                                sub(LLR, 3, [[7, W], [1, 4]]))

            for it in range(iters):
                full = (it == 0)
                lastit = (it == iters - 1)
                for c in range(CHUNKS):
                    LLR, M, LU, SG, NL = LLRs[c], Ms[c], LUs[c], SGs[c], NLs[c]
                    # scratch (tag-shared slots rotate across chunk bodies)
                    ZU  = work.tile([P, W * 12], f32, tag="ZU", name="ZU")
                    LPR = work.tile([P, W * 12], f32, tag="LPR", name="LPR")
                    LQS = work.tile([P, W * 12], f32, tag="LQS", name="LQS")
                    T6  = work.tile([P, W * 6], f32, tag="T6", name="T6")
                    S3  = work.tile([P, W * 3], f32, tag="S3", name="S3")
                    G6  = work.tile([P, W * 6], f32, tag="G6", name="G6")
                    G3  = work.tile([P, W * 3], f32, tag="G3", name="G3")
                    DM  = work.tile([P, W * 12], f32, tag="DM", name="DM")
                    SL  = work.tile([P, W * 12], f32, tag="SL", name="SL")
                    CV  = work.tile([P, W * 12], f32, tag="CV", name="CV")
                    TP  = work.tile([P, W * 2], f32, tag="TP", name="TP")

                    sl = (lambda t: t[:]) if full else dyn9
                    # phi1: lu = ln(1-z) - ln(1+z), z = exp(-|m|) clamped < 1
                    act(sl(ZU), sl(M), F.Abs)
                    act(sl(ZU), sl(ZU), F.Exp, scale=-1.0)
                    act(sl(LPR), sl(ZU), F.Ln, bias=1.0)
                    # scale/bias chosen so the argument stays >= 6e-8 even at
                    # z == 1.0 (m == +-0): keeps lu finite and strictly < 0
                    act(sl(LQS), sl(ZU), F.Ln, bias=CB2[:], scale=-0.99999988)
                    vec.tensor_tensor(sl(LU), sl(LQS), sl(LPR), Op.subtract)
                    # sign (+1 at exact zero via tiny bias)
                    act(sl(SG), sl(M), F.Sign, bias=CB[:])

                    # check sums / sign products
                    vec.tensor_tensor(T6[:], sub(LU, 0, [[12, W], [4, 3], [1, 2]]),
                                      sub(LU, 2, [[12, W], [4, 3], [1, 2]]), Op.add)
                    vec.tensor_tensor(S3[:], sub(T6, 0, [[6, W], [2, 3]]),
                                      sub(T6, 1, [[6, W], [2, 3]]), Op.add)
                    gps.tensor_tensor(G6[:], sub(SG, 0, [[12, W], [4, 3], [1, 2]]),
                                      sub(SG, 2, [[12, W], [4, 3], [1, 2]]), Op.mult)
                    gps.tensor_tensor(G3[:], sub(G6, 0, [[6, W], [2, 3]]),
                                      sub(G6, 1, [[6, W], [2, 3]]), Op.mult)

                    slg = g12 if lastit else dyn9
                    slf = (lambda t: t[:]) if lastit else dyn9
                    S3r = sub(S3, 0, [[3, W], [1, 3], [0, 4 if lastit else 3]])
                    G3r = sub(G3, 0, [[3, W], [1, 3], [0, 4 if lastit else 3]])
                    vec.tensor_tensor(slg(DM), S3r, slg(LU), Op.subtract)
                    act(slf(ZU), slf(DM), F.Exp)
                    act(slf(LPR), slf(ZU), F.Ln, bias=1.0)
                    act(slf(LQS), slf(ZU), F.Ln, bias=1.0, scale=-1.0)
                    gps.tensor_tensor(slg(SL), G3r, slg(SG), Op.mult)
                    vec.tensor_tensor(slf(DM), slf(LPR), slf(LQS), Op.subtract)
                    vec.tensor_tensor(slf(CV), slf(DM), slf(SL), Op.mult)

                    # new_llr for feedback vars v2,v5 (pairs), v4, v6
                    vec.tensor_tensor(TP[:], sub(CV, 1, [[12, W], [5, 2]]),
                                      sub(CV, 5, [[12, W], [5, 2]]), Op.add)
                    vec.tensor_tensor(sub(NL, 2, [[7, W], [3, 2]]),
                                      sub(LLR, 2, [[7, W], [3, 2]]),
                                      TP[:], Op.add)
                    vec.tensor_tensor(sub(NL, 4, [[7, W], [2, 2]]),
                                      sub(LLR, 4, [[7, W], [2, 2]]),
                                      sub(CV, 2, [[12, W], [1, 2]]), Op.add)
                    vec.tensor_tensor(sub(NL, 4, [[7, W], [2, 2]]),
                                      sub(NL, 4, [[7, W], [2, 2]]),
                                      sub(CV, 9, [[12, W], [-2, 2]]), Op.add)
                    vec.tensor_tensor(sub(NL, 6, [[7, W], [1, 1]]),
                                      sub(NL, 6, [[7, W], [1, 1]]),
                                      sub(CV, 11, [[12, W], [1, 1]]), Op.add)

                    if lastit:
                        vec.tensor_tensor(sub(NL, 0, [[7, W], [1, 2]]),
                                          sub(LLR, 0, [[7, W], [1, 2]]),
                                          sub(CV, 0, [[12, W], [4, 2]]), Op.add)
                        vec.tensor_tensor(sub(NL, 3, [[7, W], [1, 1]]),
                                          sub(LLR, 3, [[7, W], [1, 1]]),
                                          sub(CV, 8, [[12, W], [1, 1]]), Op.add)
                        nc.sync.dma_start(out=out_ap[c], in_=NL[:])
                    else:
                        # m' = new_llr - c2v for the 9 dynamic edges
                        vec.tensor_tensor(sub(M, 1, [[12, W], [1, 3]]),
                                          sub(NL, 2, [[7, W], [2, 3]]),
                                          sub(CV, 1, [[12, W], [1, 3]]), Op.subtract)
                        vec.tensor_tensor(sub(M, 9, [[12, W], [1, 3]]),
                                          sub(NL, 4, [[7, W], [1, 3]]),
                                          sub(CV, 9, [[12, W], [1, 3]]), Op.subtract)
                        vec.tensor_tensor(sub(M, 5, [[12, W], [1, 1]]),
                                          sub(NL, 2, [[7, W], [1, 1]]),
                                          sub(CV, 5, [[12, W], [1, 1]]), Op.subtract)
                        vec.tensor_tensor(sub(M, 6, [[12, W], [1, 2]]),
                                          sub(NL, 5, [[7, W], [1, 2]]),
                                          sub(CV, 6, [[12, W], [1, 2]]), Op.subtract)

    # walrus on this stack supports a single sync-wait slot per instruction.
    # Tile emits (a) redundant same-engine waits (trivially satisfied by the
    # engine's FIFO program order once the preceding updates have happened)
    # and (b) a kernel-tail SP drain waiting on the whole global clock, where
    # only the output-DMA wait is load-bearing (the per-engine drain + EVSEM
    # butterfly that follows enforces engine completion).  Strip both.
    import bass_rust
    pref = {"EngineType.DVE": "DVE_", "EngineType.Pool": "Pool_",
            "EngineType.Activation": "Activation_", "EngineType.PE": "PE_",
            "EngineType.SP": "SP_"}
    inc = {}
    for b in nc.m.functions[0].blocks:
        for i in b.instructions:
            si = i.sync_info
            if si is None:
                continue
            if len(si.on_wait) > 1:
                if type(i).__name__ == "InstDrain":
                    dma = [w for w in si.on_wait if "DMA" in w.ant_name]
                    keep_w = dma[-1:] if dma else list(si.on_wait)[:1]
                else:
                    p = pref.get(str(i.engine))
                    keep_w = [w for w in si.on_wait
                              if not (p and w.ant_name.startswith(p)
                                      and w.wait_value <= inc.get(w.ant_name, 0))]
                    assert len(keep_w) <= 1, (i.name, [(w.ant_name, w.wait_value) for w in keep_w], {k: inc.get(k) for k in [w.ant_name for w in si.on_wait]})
                i.sync_info = bass_rust.SyncInfo(on_wait=keep_w,
                                                on_update=list(si.on_update))
                si = i.sync_info
            for u in si.on_update:
                if u.update_mode == "sem-inc":
                    inc[u.ant_name] = inc.get(u.ant_name, 0) + u.update_value
    return nc


def kernel(llr, max_iters):
    llr = np.ascontiguousarray(np.asarray(llr), dtype=np.float32)
    iters = int(np.asarray(max_iters))
    B = llr.shape[0]
    if iters <= 0:
        return llr.reshape(B, 1, 7).copy()

    from concourse.bass_utils import run_bass_kernel_spmd

    Bc = B // NCORES
    key = (Bc, iters)
    if key not in _CACHE:
        _CACHE[key] = _build(Bc, iters)
    nc = _CACHE[key]

    flat = llr.reshape(B, 7)
    in_maps = [{"llr": flat[i * Bc:(i + 1) * Bc]} for i in range(NCORES)]
    res = run_bass_kernel_spmd(nc, in_maps, core_ids=list(range(NCORES)))
    out = np.concatenate([np.asarray(r["out"]) for r in res.results], axis=0)
    return out.reshape(B, 1, 7)



# revision 11
# speedup vs baseline: 1.5285x; 1.5285x over previous
"""LDPC belief-propagation kernel for Trainium2 (8 NeuronCores, data-parallel).

Tanh-product (signed) formulation of sum-product BP. Per batch row, H fixed
[3,7], 12 edges in check-major order:
  tau_e = tanh(m_e / 2)                     (signed, in (-1,1))
  u_e   = prod_{e' in c, e' != e} tau_e'    (leave-one-out product, signed)
  c2v_e = ln(1+u_e) - ln(1-u_e)             (= 2 artanh(u_e), signed)
  m'_e  = llr_v(e) + sum_{c' ni v, c' != c} c2v_{c'}
  new_llr_v = llr_v + sum_{c ni v} c2v
Signs ride inside the products, so no Abs/Sign/Exp ops are needed at all:
ACT work is 1 Tanh + 2 Ln per iteration (vs 8 ops for the phi-domain form).
The 1 +- u affines and the |u|<1 clamp fold into the Ln scale/bias
(arg >= 6e-8, matching the baseline's saturation behaviour).
Edges of degree-1 variables (e0,e4,e8 = first edge of each check) carry
constant messages == llr: their tau is computed once; per-iteration work
covers only the 9 dynamic edges, full 12-edge c2v only on the last iteration.
Batch is split into chunks so ACT/DVE/Pool/DMA pipeline across chunks.
"""

import numpy as np

_CACHE = {}

NCORES = 8
P = 128      # partitions
CHUNKS = 2   # batch sub-chunks per core (pipeline depth)


def _build(Bc, iters):
    import contextlib

    import concourse.bass as bass
    import concourse.tile as tile
    from concourse import mybir
    from concourse.alu_op_type import AluOpType as Op

    F = mybir.ActivationFunctionType
    W = Bc // P // CHUNKS  # free columns per partition per chunk
    f32 = mybir.dt.float32

    nc = bass.Bass("TRN2", target_bir_lowering=False, debug=False,
                   num_devices=1)
    llr_d = nc.dram_tensor("llr", [Bc, 7], f32, kind="ExternalInput")
    out_d = nc.dram_tensor("out", [Bc, 7], f32, kind="ExternalOutput")

    def sub(t, off, dims):
        a = t[:] if callable(getattr(t, "__getitem__", None)) else t
        return bass.AP(tensor=a.tensor, offset=a.offset + off,
                       ap=[list(a.ap[0])] + [list(d) for d in dims])

    with tile.TileContext(nc) as tc:
        ctx = contextlib.ExitStack()
        with ctx:
            keep = ctx.enter_context(tc.tile_pool(name="keep", bufs=1))
            work = ctx.enter_context(tc.tile_pool(name="work", bufs=2))

            def K(name, k):
                return keep.tile([P, W * k], f32, tag=name, name=name)

            CB2 = keep.tile([P, 1], f32, tag="CB2", name="CB2")
            nc.vector.memset(CB2, 0.99999994)

            # per-chunk persistent state
            LLRs = [K(f"LLR{c}", 7) for c in range(CHUNKS)]
            LLEs = [K(f"LLE{c}", 12) for c in range(CHUNKS)]   # llr scattered to edges
            TAUs = [K(f"TAU{c}", 12) for c in range(CHUNKS)]   # tanh(m/2) per edge
            NLs  = [K(f"NL{c}", 7) for c in range(CHUNKS)]

            act = nc.scalar.activation
            vec = nc.vector
            gps = nc.gpsimd

            def dyn9(t):
                return sub(t, 1, [[12, W], [4, 3], [1, 3]])

            llr_ap = llr_d.ap().rearrange("(c p w) v -> c p (w v)", c=CHUNKS, p=P)
            out_ap = out_d.ap().rearrange("(c p w) v -> c p (w v)", c=CHUNKS, p=P)

            SC = 0.99999988

            for c in range(CHUNKS):
                LLR, LLE, TAU = LLRs[c], LLEs[c], TAUs[c]
                nc.sync.dma_start(out=LLR[:], in_=llr_ap[c])
                # scatter llr to edge slots: LLE[e] = llr[v(e)]
                vec.tensor_copy(sub(LLE, 0, [[12, W], [1, 4]]),
                                sub(LLR, 0, [[7, W], [2, 4]]))
                vec.tensor_copy(sub(LLE, 4, [[12, W], [1, 2]]),
                                sub(LLR, 1, [[7, W], [1, 2]]))
                vec.tensor_copy(sub(LLE, 6, [[12, W], [1, 2]]),
                                sub(LLR, 5, [[7, W], [1, 2]]))
                vec.tensor_copy(sub(LLE, 8, [[12, W], [1, 4]]),
                                sub(LLR, 3, [[7, W], [1, 4]]))
                act(TAU[:], LLE[:], F.Tanh, scale=0.5)

            for it in range(iters):
                lastit = (it == iters - 1)
                # scratch tiles (tag-shared slots rotate across chunk bodies)
                ABs, Us, LNPs, LNMs, CVs, MPs, Ss, TPs, Xs = ([], [], [], [],
                                                              [], [], [], [], [])
                for c in range(CHUNKS):
                    ABs.append(work.tile([P, W * 6], f32, tag="AB", name="AB"))
                    Us.append(work.tile([P, W * 12], f32, tag="U", name="U"))
                    LNPs.append(work.tile([P, W * 12], f32, tag="LNP", name="LNP"))
                    LNMs.append(work.tile([P, W * 12], f32, tag="LNM", name="LNM"))
                    CVs.append(work.tile([P, W * 12], f32, tag="CV", name="CV"))
                    if not lastit:
                        MPs.append(work.tile([P, W * 12], f32, tag="MP", name="MP"))
                        Ss.append(work.tile([P, W * 3], f32, tag="S", name="S"))
                    else:
                        TPs.append(work.tile([P, W * 2], f32, tag="TP", name="TP"))
                        Xs.append(work.tile([P, W * 2], f32, tag="X", name="X"))

                # products: a_c = tau0*tau1, b_c = tau2*tau3 per check,
                # then leave-one-out u_e
                for c in range(CHUNKS):
                    TAU, AB, U = TAUs[c], ABs[c], Us[c]
                    gps.tensor_tensor(sub(AB, 0, [[6, W], [2, 3], [1, 2]]),
                                      sub(TAU, 0, [[12, W], [4, 3], [2, 2]]),
                                      sub(TAU, 1, [[12, W], [4, 3], [2, 2]]),
                                      Op.mult)
                    gps.tensor_tensor(sub(U, 1, [[12, W], [4, 3]]),
                                      sub(TAU, 0, [[12, W], [4, 3]]),
                                      sub(AB, 1, [[6, W], [2, 3]]),
                                      Op.mult)
                    gps.tensor_tensor(sub(U, 2, [[12, W], [4, 3]]),
                                      sub(AB, 0, [[6, W], [2, 3]]),
                                      sub(TAU, 3, [[12, W], [4, 3]]),
                                      Op.mult)
                    gps.tensor_tensor(sub(U, 3, [[12, W], [4, 3]]),
                                      sub(AB, 0, [[6, W], [2, 3]]),
                                      sub(TAU, 2, [[12, W], [4, 3]]),
                                      Op.mult)
                    if lastit:
                        gps.tensor_tensor(sub(U, 0, [[12, W], [4, 3]]),
                                          sub(TAU, 1, [[12, W], [4, 3]]),
                                          sub(AB, 1, [[6, W], [2, 3]]),
                                          Op.mult)

                # c2v = ln(1+u) - ln(1-u), clamps folded into Ln scale/bias
                sl = (lambda t: t[:]) if lastit else dyn9
                for c in range(CHUNKS):
                    U, LNP, LNM = Us[c], LNPs[c], LNMs[c]
                    act(sl(LNP), sl(U), F.Ln, bias=CB2[:], scale=SC)
                    act(sl(LNM), sl(U), F.Ln, bias=CB2[:], scale=-SC)
                for c in range(CHUNKS):
                    vec.tensor_tensor(sl(CVs[c]), sl(LNPs[c]), sl(LNMs[c]),
                                      Op.subtract)

                if not lastit:
                    for c in range(CHUNKS):
                        LLE, CV, MP, S = LLEs[c], CVs[c], MPs[c], Ss[c]
                        # deg-2 vars: m'[e] = LLE[e] + c2v[partner(e)]
                        # pairs (e1,e5),(e6,e10) in one op; (e2,e9) in another
                        vec.tensor_tensor(sub(MP, 1, [[12, W], [5, 2], [4, 2]]),
                                          sub(LLE, 1, [[12, W], [5, 2], [4, 2]]),
                                          sub(CV, 5, [[12, W], [5, 2], [-4, 2]]),
                                          Op.add)
                        vec.tensor_tensor(sub(MP, 2, [[12, W], [7, 2]]),
                                          sub(LLE, 2, [[12, W], [7, 2]]),
                                          sub(CV, 9, [[12, W], [-7, 2]]),
                                          Op.add)
                        # v6 (deg 3): NL6 = llr6 + c2v3 + c2v7 + c2v11
                        vec.tensor_tensor(sub(S, 0, [[3, W]]),
                                          sub(CV, 3, [[12, W]]),
                                          sub(CV, 7, [[12, W]]), Op.add)
                        gps.tensor_tensor(sub(S, 1, [[3, W]]),
                                          sub(LLE, 3, [[12, W]]),
                                          sub(CV, 11, [[12, W]]), Op.add)
                        vec.tensor_tensor(sub(S, 2, [[3, W]]),
                                          sub(S, 0, [[3, W]]),
                                          sub(S, 1, [[3, W]]), Op.add)
                        vec.tensor_tensor(sub(MP, 3, [[12, W], [4, 3]]),
                                          sub(S, 2, [[3, W], [0, 3]]),
                                          sub(CV, 3, [[12, W], [4, 3]]),
                                          Op.subtract)
                    for c in range(CHUNKS):
                        act(dyn9(TAUs[c]), dyn9(MPs[c]), F.Tanh, scale=0.5)
                else:
                    for c in range(CHUNKS):
                        LLR, CV, NL, TP, X = (LLRs[c], CVs[c], NLs[c],
                                              TPs[c], Xs[c])
                        # v2/v5 via paired sums
                        vec.tensor_tensor(sub(TP, 0, [[2, W], [1, 2]]),
                                          sub(CV, 1, [[12, W], [5, 2]]),
                                          sub(CV, 5, [[12, W], [5, 2]]), Op.add)
                        vec.tensor_tensor(sub(NL, 2, [[7, W], [3, 2]]),
                                          sub(LLR, 2, [[7, W], [3, 2]]),
                                          sub(TP, 0, [[2, W], [1, 2]]), Op.add)
                        # v4/v6
                        vec.tensor_tensor(sub(X, 0, [[2, W], [1, 2]]),
                                          sub(CV, 2, [[12, W], [1, 2]]),
                                          sub(CV, 9, [[12, W], [2, 2]]), Op.add)
                        vec.tensor_tensor(sub(NL, 4, [[7, W], [2, 2]]),
                                          sub(LLR, 4, [[7, W], [2, 2]]),
                                          sub(X, 0, [[2, W], [1, 2]]), Op.add)
                        vec.tensor_tensor(sub(NL, 6, [[7, W]]),
                                          sub(NL, 6, [[7, W]]),
                                          sub(CV, 7, [[12, W]]), Op.add)
                        # deg-1 vars v0,v1,v3
                        vec.tensor_tensor(sub(NL, 0, [[7, W], [1, 2]]),
                                          sub(LLR, 0, [[7, W], [1, 2]]),
                                          sub(CV, 0, [[12, W], [4, 2]]), Op.add)
                        vec.tensor_tensor(sub(NL, 3, [[7, W]]),
                                          sub(LLR, 3, [[7, W]]),
                                          sub(CV, 8, [[12, W]]), Op.add)
                        nc.sync.dma_start(out=out_ap[c], in_=NLs[c][:])

    _reduce_syncs(nc)
    return nc


def _reduce_syncs(nc):
    """walrus on this stack supports a single sync-wait slot per instruction,
    but Tile emits every data/anti-dependency as its own wait.  Most are
    transitively implied: if I waits on sem s >= v, and the instruction that
    raised s to v had itself (directly or transitively) waited on t >= w,
    then s >= v implies t >= w at any later time.  Compute that happens-before
    closure with per-engine vector clocks (engines issue and complete
    in-order; sem updates fire at completion) and keep, per instruction, a
    single wait that covers all the others."""
    import bass_rust

    eng_vc = {}     # engine -> {sem: known-reached value}
    sem_hist = {}   # sem -> [(value_after, snapshot_clock)] in program order
    sem_total = {}
    multi = []

    # Semaphores with any non-increment update (barrier gather sems use
    # sem-sub) are non-monotonic: their waits must be kept verbatim and they
    # cannot participate in happens-before reasoning.
    nonmono = set()
    for b in nc.m.functions[0].blocks:
        for i in b.instructions:
            si = i.sync_info
            if si is not None:
                for u in si.on_update:
                    if u.update_mode != "sem-inc":
                        nonmono.add(u.ant_name)

    def snap_at(sem, v):
        for val, snapshot in sem_hist.get(sem, ()):
            if val >= v:
                return snapshot
        return None

    for b in nc.m.functions[0].blocks:
        for i in b.instructions:
            si = i.sync_info
            eng = str(i.engine)
            vc = eng_vc.setdefault(eng, {})
            if si is not None and si.on_wait:
                byname = {}
                fixed = []
                for w in si.on_wait:
                    if w.ant_name in nonmono:
                        fixed.append(w)
                        continue
                    o = byname.get(w.ant_name)
                    if o is None or o.wait_value < w.wait_value:
                        byname[w.ant_name] = w
                pend = [w for w in byname.values()
                        if vc.get(w.ant_name, 0) < w.wait_value]
                keep = pend
                if type(i).__name__ == "InstDrain" and len(fixed) + len(pend) > 1:
                    # kernel-tail drain: only the output-DMA wait is
                    # load-bearing (the per-engine drain + EVSEM butterfly
                    # that follows enforces engine completion)
                    dma = [w for w in fixed + pend if "DMA" in w.ant_name]
                    if dma:
                        fixed = []
                        pend = dma[-1:]
                        keep = pend
                if len(pend) > 1:
                    for w in pend:
                        s = snap_at(w.ant_name, w.wait_value)
                        if s is None:
                            continue
                        if all(w2 is w
                               or max(vc.get(w2.ant_name, 0),
                                      s.get(w2.ant_name, 0)) >= w2.wait_value
                               for w2 in pend):
                            keep = [w]
                            break
                for w in keep:
                    s = snap_at(w.ant_name, w.wait_value)
                    if s:
                        for k, v2 in s.items():
                            if vc.get(k, 0) < v2:
                                vc[k] = v2
                    if vc.get(w.ant_name, 0) < w.wait_value:
                        vc[w.ant_name] = w.wait_value
                keep = fixed + keep
                if len(keep) > 1:
                    multi.append((i.name, eng,
                                  [(w.ant_name, w.wait_value) for w in keep]))
                i.sync_info = bass_rust.SyncInfo(on_wait=keep,
                                                 on_update=list(si.on_update))
                si = i.sync_info
            if si is not None:
                for u in si.on_update:
                    if u.update_mode == "sem-inc" and u.ant_name not in nonmono:
                        tot = sem_total.get(u.ant_name, 0) + u.update_value
                        sem_total[u.ant_name] = tot
                        vc[u.ant_name] = tot
                        snapshot = dict(vc)
                        sem_hist.setdefault(u.ant_name, []).append(
                            (tot, snapshot))
    assert not multi, ("irreducible multi-wait instructions", multi[:8])


def kernel(llr, max_iters):
    llr = np.ascontiguousarray(np.asarray(llr), dtype=np.float32)
    iters = int(np.asarray(max_iters))
    B = llr.shape[0]
    if iters <= 0:
        return llr.reshape(B, 1, 7).copy()

    from concourse.bass_utils import run_bass_kernel_spmd

    Bc = B // NCORES
    key = (Bc, iters)
    if key not in _CACHE:
        _CACHE[key] = _build(Bc, iters)
    nc = _CACHE[key]

    flat = llr.reshape(B, 7)
    in_maps = [{"llr": flat[i * Bc:(i + 1) * Bc]} for i in range(NCORES)]
    res = run_bass_kernel_spmd(nc, in_maps, core_ids=list(range(NCORES)))
    out = np.concatenate([np.asarray(r["out"]) for r in res.results], axis=0)
    return out.reshape(B, 1, 7)


# revision 21
# speedup vs baseline: 2.3320x; 1.5257x over previous
"""LDPC belief-propagation kernel for Trainium2 (8 NeuronCores, data-parallel).

Tanh-product (signed) formulation of sum-product BP. Per batch row, H fixed
[3,7], 12 edges in check-major order:
  tau_e = tanh(m_e / 2)                     (signed, in (-1,1))
  u_e   = prod_{e' in c, e' != e} tau_e'    (leave-one-out product, signed)
  c2v_e = ln(1+u_e) - ln(1-u_e)             (= 2 artanh(u_e), signed)
  m'_e  = llr_v(e) + sum_{c' ni v, c' != c} c2v_{c'}
  new_llr_v = llr_v + sum_{c ni v} c2v
Signs ride inside the products, so no Abs/Sign/Exp ops are needed at all:
ACT work is 1 Tanh + 2 Ln per iteration (vs 8 ops for the phi-domain form).
The 1 +- u affines and the |u|<1 clamp fold into the Ln scale/bias
(arg >= 6e-8, matching the baseline's saturation behaviour).
Edges of degree-1 variables (e0,e4,e8 = first edge of each check) carry
constant messages == llr: their tau is computed once; per-iteration work
covers only the 9 dynamic edges, full 12-edge c2v only on the last iteration.
Batch is split into chunks so ACT/DVE/Pool/DMA pipeline across chunks.
"""

import numpy as np

_CACHE = {}

NCORES = 8
P = 128      # partitions
CHUNKS = 2   # batch sub-chunks per core (pipeline depth)


def _build(Bc, iters):
    import contextlib

    import concourse.bass as bass
    import concourse.tile as tile
    from concourse import mybir
    from concourse.alu_op_type import AluOpType as Op

    F = mybir.ActivationFunctionType
    W = Bc // P // CHUNKS  # free columns per partition per chunk
    f32 = mybir.dt.float32

    nc = bass.Bass("TRN2", target_bir_lowering=False, debug=False,
                   num_devices=1)
    llr_d = nc.dram_tensor("llr", [Bc, 7], f32, kind="ExternalInput")
    out_d = nc.dram_tensor("out", [Bc, 7], f32, kind="ExternalOutput")

    def sub(t, off, dims):
        a = t[:] if callable(getattr(t, "__getitem__", None)) else t
        return bass.AP(tensor=a.tensor, offset=a.offset + off,
                       ap=[list(a.ap[0])] + [list(d) for d in dims])

    with tile.TileContext(nc) as tc:
        ctx = contextlib.ExitStack()
        with ctx:
            keep = ctx.enter_context(tc.tile_pool(name="keep", bufs=1))
            work = ctx.enter_context(tc.tile_pool(name="work", bufs=2))

            def K(name, k):
                return keep.tile([P, W * k], f32, tag=name, name=name)

            CB2 = keep.tile([P, 1], f32, tag="CB2", name="CB2")
            nc.vector.memset(CB2, 0.99999994)

            # per-chunk persistent state
            LLRs = [K(f"LLR{c}", 7) for c in range(CHUNKS)]
            LLEs = [K(f"LLE{c}", 12) for c in range(CHUNKS)]   # llr scattered to edges
            TAUs = [K(f"TAU{c}", 12) for c in range(CHUNKS)]   # tanh(m/2) per edge
            NLs  = [K(f"NL{c}", 7) for c in range(CHUNKS)]

            act = nc.scalar.activation
            vec = nc.vector
            gps = nc.gpsimd

            def dyn9(t):
                return sub(t, 1, [[12, W], [4, 3], [1, 3]])

            llr_ap = llr_d.ap().rearrange("(c p w) v -> c p (w v)", c=CHUNKS, p=P)
            out_ap = out_d.ap().rearrange("(c p w) v -> c p (w v)", c=CHUNKS, p=P)

            SC = 0.99999988

            # input DMA split into half-chunks so edge-scatter + init tanh
            # start as soon as the first half lands.
            H = W // 2
            for c in range(CHUNKS):
                LLR, LLE, TAU = LLRs[c], LLEs[c], TAUs[c]
                a = llr_ap[c]
                for h in range(2):
                    lo, eo, vo = 12 * H * h, 12 * H * h, 7 * H * h
                    nc.sync.dma_start(
                        out=sub(LLR, 7 * H * h, [[1, 7 * H]]),
                        in_=bass.AP(tensor=a.tensor,
                                    offset=a.offset + 7 * H * h,
                                    ap=[list(a.ap[0])] + [[1, 7 * H]]))
                    # scatter llr to edge slots: LLE[e] = llr[v(e)]
                    vec.tensor_copy(sub(LLE, eo + 0, [[12, H], [1, 4]]),
                                    sub(LLR, vo + 0, [[7, H], [2, 4]]))
                    gps.tensor_copy(sub(LLE, eo + 4, [[12, H], [1, 2]]),
                                    sub(LLR, vo + 1, [[7, H], [1, 2]]))
                    gps.tensor_copy(sub(LLE, eo + 6, [[12, H], [1, 2]]),
                                    sub(LLR, vo + 5, [[7, H], [1, 2]]))
                    vec.tensor_copy(sub(LLE, eo + 8, [[12, H], [1, 4]]),
                                    sub(LLR, vo + 3, [[7, H], [1, 4]]))
                    act(sub(TAU, eo, [[1, 12 * H]]),
                        sub(LLE, eo, [[1, 12 * H]]), F.Tanh, scale=0.5)

            for it in range(iters):
                lastit = (it == iters - 1)
                sl = (lambda t: t[:]) if lastit else dyn9
                # full per-chunk bodies so the ACT FIFO order is
                # [Ln,Ln,Tanh] per chunk — lets the two chunks anti-phase
                for c in range(CHUNKS):
                    TAU, LLE = TAUs[c], LLEs[c]
                    AB = work.tile([P, W * 6], f32, tag="AB", name="AB")
                    U = work.tile([P, W * 12], f32, tag="U", name="U")
                    LNP = work.tile([P, W * 12], f32, tag="LNP", name="LNP")
                    LNM = work.tile([P, W * 12], f32, tag="LNM", name="LNM")
                    CV = work.tile([P, W * 12], f32, tag="CV", name="CV")
                    # products: a_c = tau0*tau1, b_c = tau2*tau3 per check,
                    # then leave-one-out u_e (split DVE/Pool to shorten chain)
                    vec.tensor_tensor(sub(AB, 0, [[6, W], [2, 3], [1, 2]]),
                                      sub(TAU, 0, [[12, W], [4, 3], [2, 2]]),
                                      sub(TAU, 1, [[12, W], [4, 3], [2, 2]]),
                                      Op.mult)
                    gps.tensor_tensor(sub(U, 1, [[12, W], [4, 3]]),
                                      sub(TAU, 0, [[12, W], [4, 3]]),
                                      sub(AB, 1, [[6, W], [2, 3]]),
                                      Op.mult)
                    vec.tensor_tensor(sub(U, 2, [[12, W], [4, 3], [1, 2]]),
                                      sub(AB, 0, [[6, W], [2, 3], [0, 2]]),
                                      sub(TAU, 3, [[12, W], [4, 3], [-1, 2]]),
                                      Op.mult)
                    if lastit:
                        gps.tensor_tensor(sub(U, 0, [[12, W], [4, 3]]),
                                          sub(TAU, 1, [[12, W], [4, 3]]),
                                          sub(AB, 1, [[6, W], [2, 3]]),
                                          Op.mult)
                    # c2v = ln(1+u) - ln(1-u), clamp folded into Ln scale/bias
                    act(sl(LNP), sl(U), F.Ln, bias=CB2[:], scale=SC)
                    act(sl(LNM), sl(U), F.Ln, bias=CB2[:], scale=-SC)

                    if not lastit:
                        MP = work.tile([P, W * 12], f32, tag="MP", name="MP")
                        S = work.tile([P, W * 3], f32, tag="S", name="S")
                        # c2v split: deg-2 slots {1,2,5,6,9,10} on DVE,
                        # v6 slots {3,7,11} on Pool — parallel after LNM
                        vec.tensor_tensor(sub(CV, 1, [[12, W], [4, 3], [1, 2]]),
                                          sub(LNP, 1, [[12, W], [4, 3], [1, 2]]),
                                          sub(LNM, 1, [[12, W], [4, 3], [1, 2]]),
                                          Op.subtract)
                        gps.tensor_tensor(sub(CV, 3, [[12, W], [4, 3]]),
                                          sub(LNP, 3, [[12, W], [4, 3]]),
                                          sub(LNM, 3, [[12, W], [4, 3]]),
                                          Op.subtract)
                        # deg-2 vars: m'[e] = LLE[e] + c2v[partner(e)]
                        gps.tensor_tensor(sub(MP, 1, [[12, W], [5, 2], [4, 2]]),
                                          sub(LLE, 1, [[12, W], [5, 2], [4, 2]]),
                                          sub(CV, 5, [[12, W], [5, 2], [-4, 2]]),
                                          Op.add)
                        vec.tensor_tensor(sub(MP, 2, [[12, W], [7, 2]]),
                                          sub(LLE, 2, [[12, W], [7, 2]]),
                                          sub(CV, 9, [[12, W], [-7, 2]]),
                                          Op.add)
                        # v6 (deg 3): m'[e] = llr6 + sum of other two c2v
                        vec.tensor_tensor(sub(S, 0, [[3, W], [1, 2]]),
                                          sub(LLE, 3, [[12, W], [4, 2]]),
                                          sub(CV, 7, [[12, W], [4, 2]]),
                                          Op.add)
                        gps.tensor_tensor(sub(S, 2, [[3, W]]),
                                          sub(LLE, 11, [[12, W]]),
                                          sub(CV, 3, [[12, W]]), Op.add)
                        vec.tensor_tensor(sub(MP, 3, [[12, W], [4, 2]]),
                                          sub(S, 0, [[3, W], [1, 2]]),
                                          sub(CV, 11, [[12, W], [-8, 2]]),
                                          Op.add)
                        vec.tensor_tensor(sub(MP, 11, [[12, W]]),
                                          sub(S, 2, [[3, W]]),
                                          sub(CV, 7, [[12, W]]), Op.add)
                        act(dyn9(TAU), dyn9(MP), F.Tanh, scale=0.5)
                    else:
                        NL = NLs[c]
                        TP = work.tile([P, W * 2], f32, tag="TP", name="TP")
                        X = work.tile([P, W * 2], f32, tag="X", name="X")
                        vec.tensor_tensor(sl(CV), sl(LNP), sl(LNM),
                                          Op.subtract)
                        # v2/v5 via paired sums (llr read from LLE slots)
                        vec.tensor_tensor(sub(TP, 0, [[2, W], [1, 2]]),
                                          sub(CV, 1, [[12, W], [5, 2]]),
                                          sub(CV, 5, [[12, W], [5, 2]]), Op.add)
                        vec.tensor_tensor(sub(NL, 2, [[7, W], [3, 2]]),
                                          sub(LLE, 1, [[12, W], [5, 2]]),
                                          sub(TP, 0, [[2, W], [1, 2]]), Op.add)
                        # v4/v6
                        gps.tensor_tensor(sub(X, 0, [[2, W], [1, 2]]),
                                          sub(CV, 2, [[12, W], [1, 2]]),
                                          sub(CV, 9, [[12, W], [2, 2]]), Op.add)
                        gps.tensor_tensor(sub(NL, 4, [[7, W], [2, 2]]),
                                          sub(LLE, 2, [[12, W], [1, 2]]),
                                          sub(X, 0, [[2, W], [1, 2]]), Op.add)
                        gps.tensor_tensor(sub(NL, 6, [[7, W]]),
                                          sub(NL, 6, [[7, W]]),
                                          sub(CV, 7, [[12, W]]), Op.add)
                        # deg-1 vars v0,v1,v3
                        vec.tensor_tensor(sub(NL, 0, [[7, W], [1, 2]]),
                                          sub(LLE, 0, [[12, W], [4, 2]]),
                                          sub(CV, 0, [[12, W], [4, 2]]), Op.add)
                        vec.tensor_tensor(sub(NL, 3, [[7, W]]),
                                          sub(LLE, 8, [[12, W]]),
                                          sub(CV, 8, [[12, W]]), Op.add)
                        nc.sync.dma_start(out=out_ap[c], in_=NL[:])

    _reduce_syncs(nc)
    return nc


def _reduce_syncs(nc):
    """walrus on this stack supports a single sync-wait slot per instruction,
    but Tile emits every data/anti-dependency as its own wait.  Most are
    transitively implied: if I waits on sem s >= v, and the instruction that
    raised s to v had itself (directly or transitively) waited on t >= w,
    then s >= v implies t >= w at any later time.  Compute that happens-before
    closure with per-engine vector clocks (engines issue and complete
    in-order; sem updates fire at completion) and keep, per instruction, a
    single wait that covers all the others."""
    import bass_rust

    eng_vc = {}     # engine -> {sem: known-reached value}
    sem_hist = {}   # sem -> [(value_after, snapshot_clock)] in program order
    sem_total = {}
    multi = []
    es_n = [0]
    inserts = []    # (block, index, new_instruction)

    # Semaphores with any non-increment update (barrier gather sems use
    # sem-sub) are non-monotonic: their waits must be kept verbatim and they
    # cannot participate in happens-before reasoning.
    nonmono = set()
    for b in nc.m.functions[0].blocks:
        for i in b.instructions:
            si = i.sync_info
            if si is not None:
                for u in si.on_update:
                    if u.update_mode != "sem-inc":
                        nonmono.add(u.ant_name)

    def snap_at(sem, v):
        for val, snapshot in sem_hist.get(sem, ()):
            if val >= v:
                return snapshot
        return None

    for b in nc.m.functions[0].blocks:
        for idx, i in enumerate(b.instructions):
            si = i.sync_info
            eng = str(i.engine)
            vc = eng_vc.setdefault(eng, {})
            if si is not None and si.on_wait:
                byname = {}
                fixed = []
                for w in si.on_wait:
                    if w.ant_name in nonmono:
                        fixed.append(w)
                        continue
                    o = byname.get(w.ant_name)
                    if o is None or o.wait_value < w.wait_value:
                        byname[w.ant_name] = w
                pend = [w for w in byname.values()
                        if vc.get(w.ant_name, 0) < w.wait_value]
                keep = pend
                if type(i).__name__ == "InstDrain" and len(fixed) + len(pend) > 1:
                    # kernel-tail drain: only the output-DMA wait is
                    # load-bearing (the per-engine drain + EVSEM butterfly
                    # that follows enforces engine completion)
                    dma = [w for w in fixed + pend if "DMA" in w.ant_name]
                    if dma:
                        fixed = []
                        pend = dma[-1:]
                        keep = pend
                if len(pend) > 1:
                    for w in pend:
                        s = snap_at(w.ant_name, w.wait_value)
                        if s is None:
                            continue
                        if all(w2 is w
                               or max(vc.get(w2.ant_name, 0),
                                      s.get(w2.ant_name, 0)) >= w2.wait_value
                               for w2 in pend):
                            keep = [w]
                            break
                for w in keep:
                    s = snap_at(w.ant_name, w.wait_value)
                    if s:
                        for k, v2 in s.items():
                            if vc.get(k, 0) < v2:
                                vc[k] = v2
                    if vc.get(w.ant_name, 0) < w.wait_value:
                        vc[w.ant_name] = w.wait_value
                keep = fixed + keep
                if len(keep) > 1 and type(i).__name__ != "InstDrain":
                    # walrus supports one wait slot per instruction: spill
                    # extra waits onto same-engine EventSemaphore no-ops
                    # (engines issue in order, so a satisfied wait on the
                    # preceding ES guarantees it for this instruction too)
                    for w in keep[:-1]:
                        es_n[0] += 1
                        es = bass_rust.InstEventSemaphore(
                            name=f"ESW-{es_n[0]}", engine=i.engine)
                        es.sync_info = bass_rust.SyncInfo(
                            on_wait=[w], on_update=[])
                        inserts.append((b, idx, es))
                    keep = keep[-1:]
                if len(keep) > 1:
                    multi.append((i.name, eng,
                                  [(w.ant_name, w.wait_value) for w in keep]))
                i.sync_info = bass_rust.SyncInfo(on_wait=keep,
                                                 on_update=list(si.on_update))
                si = i.sync_info
            if si is not None:
                for u in si.on_update:
                    if u.update_mode == "sem-inc" and u.ant_name not in nonmono:
                        tot = sem_total.get(u.ant_name, 0) + u.update_value
                        sem_total[u.ant_name] = tot
                        vc[u.ant_name] = tot
                        snapshot = dict(vc)
                        sem_hist.setdefault(u.ant_name, []).append(
                            (tot, snapshot))
    assert not multi, ("irreducible multi-wait instructions", multi[:8])
    # apply ES insertions (descending index so positions stay valid)
    by_block = {}
    for b, idx, es in inserts:
        by_block.setdefault(id(b), (b, []))[1].append((idx, es))
    for b, items in by_block.values():
        insts = list(b.instructions)
        for idx, es in sorted(items, reverse=True, key=lambda t: t[0]):
            insts.insert(idx, es)
        b.instructions = insts


def kernel(llr, max_iters):
    llr = np.ascontiguousarray(np.asarray(llr), dtype=np.float32)
    iters = int(np.asarray(max_iters))
    B = llr.shape[0]
    if iters <= 0:
        return llr.reshape(B, 1, 7).copy()

    from concourse.bass_utils import run_bass_kernel_spmd

    Bc = B // NCORES
    key = (Bc, iters)
    if key not in _CACHE:
        _CACHE[key] = _build(Bc, iters)
    nc = _CACHE[key]

    flat = llr.reshape(B, 7)
    in_maps = [{"llr": flat[i * Bc:(i + 1) * Bc]} for i in range(NCORES)]
    res = run_bass_kernel_spmd(nc, in_maps, core_ids=list(range(NCORES)))
    out = np.concatenate([np.asarray(r["out"]) for r in res.results], axis=0)
    return out.reshape(B, 1, 7)
